# revision 1
# baseline (speedup 1.0000x reference)
"""Trainium2 Bass kernel for nn_ContrastiveLoss (SimCLR NT-Xent) — v2.

Math (reference):
    reps = concat(zjs, zis)            # [8192, 128]
    rn = reps / ||reps||               # row-normalized
    sim = rn @ rn.T                    # [8192, 8192]
    den_i = sum_{j != i} exp(sim[i,j]/tau);  pos_i = sim[i, i+-B]
    CE = sum_i (log den_i - pos_i/tau);  pt = sum_i exp(pos_i/tau)/den_i
    loss = CE/N + B*(1/B - pt/(N*(N-1)))

Distribution: data-parallel over rows, 1024 rows/core, each core holding a
column-rotated copy of reps so the SPMD program is identical on all cores.

v2 vs the 139us v1:
  * all-bf16 datapath: host casts to bf16; the gram runs the PE at
    1 cycle/col (fp32 was 4) and DVE elementwise ops hit the 2x 16-bit mode.
  * lhsT pre-scaled by alpha = 10*log2(e): psum sim tiles hold t = alpha*s,
    so exp(10 s) = 2^t = ACT Exp with scale=ln2 — or a bit-trick 2^t on DVE.
  * exp work SPLIT across engines — ACT Exp alone is a hard 55us floor
    (0.83 ns/elem * 65536 elem-columns). DVE takes tiles via the Schraudolph
    trick (int32(t*2^23 + magic) bitcast to f32 gives 2^t to ~2-5% per
    element, mean-calibrated so the 8192-term den sums see ~0.05% error);
    GPSIMD optionally takes tiles via software vpowf.
  * no on-device tail: per-(m,nb) row-sum partials and the positive
    diagonals ship to the host, which does the tiny log/exp scalar tail in
    f64 along with the inter-core reduction it already owned (no Ln table
    load, no final transpose/matmul choreography).
"""
import os

os.environ.setdefault("JAX_COMPILATION_CACHE_DIR", "/root/jax_bass_cache")

import numpy as np
import ml_dtypes
from contextlib import ExitStack

import concourse.bass as bass
import concourse.tile as tile
from concourse import mybir
from concourse.bass_utils import run_bass_kernel_spmd
from concourse.vector_clock import ScopedClock

# ---------------------------------------------------------------------------
# Workaround for walrus CoreV2/V3 "Too many sync wait commands": split sem
# waits so no instruction carries more than one.
# ---------------------------------------------------------------------------
_MAX_WAITS = int(os.environ.get("BASS_MAX_WAITS", "1"))
_orig_commit = tile.TileContext._commit_instruction


def _split_waits(nc, inst):
    si = getattr(inst, "sync_info", None)
    if si is None:
        return []
    waits = list(si.on_wait)
    if len(waits) <= _MAX_WAITS:
        return []
    nops = []
    excess, keep = waits[:-_MAX_WAITS], waits[-_MAX_WAITS:]
    for i in range(0, len(excess), _MAX_WAITS):
        nops.append(
            mybir.InstNoOp(
                name=nc.get_next_instruction_name(),
                engine=inst.engine,
                bass_nofuse=True,
                sync_info=mybir.SyncInfo(
                    on_wait=excess[i : i + _MAX_WAITS], on_update=[]
                ),
            )
        )
    inst.sync_info = mybir.SyncInfo(on_wait=keep, on_update=list(si.on_update))
    return nops


def _patched_commit(self, inst, lazy_reg_writes=True):
    try:
        nops = _split_waits(self.nc, inst)
    except Exception:
        nops = []
    for nop in nops:
        _orig_commit(self, nop)
    return _orig_commit(self, inst, lazy_reg_writes)


def _patched_drain_and_barrier(self, tick_clock, wait_clock):
    nc = self.nc
    probe = mybir.InstNoOp(
        name=nc.get_next_instruction_name(),
        engine=mybir.EngineType.SP,
        bass_nofuse=True,
    )
    wait_clock.add_sem_waits(probe, ScopedClock({None: tick_clock.global_clock}))
    si = probe.sync_info
    waits = list(si.on_wait) if si is not None else []
    for i in range(0, len(waits), _MAX_WAITS):
        nop = nc.sync.nop(nofuse=True)
        nop.ins.sync_info = mybir.SyncInfo(
            on_wait=waits[i : i + _MAX_WAITS], on_update=[]
        )
    nc.sync.drain()
    nc.all_engine_barrier()
    assert self.sems is not None
    popped = nc._tile_sem_poison_stack.pop()
    assert popped is self._sem_poison
    nc.clear_and_free_semaphores(list(self.sems.allocated().values()))
    nc.all_engine_barrier()


tile.TileContext._commit_instruction = _patched_commit
tile.TileContext._drain_and_barrier = _patched_drain_and_barrier

# ---------------------------------------------------------------------------
# Content-hashed NEFF cache
# ---------------------------------------------------------------------------
import hashlib
import shutil

_NEFF_CACHE_DIR = "/root/.bass_neff_cache"

import concourse.bass_utils as _bass_utils
import concourse.bass2jax as _bass2jax

_orig_compile_bir_kernel = _bass_utils.compile_bir_kernel


_LDW_OPT = os.environ.get("BASS_LDW_OPT", "0") == "1"
_orig_run_command = _bass_utils.run_command


def _ldwopt_run_command(cmd, *a, **kw):
    if _LDW_OPT and isinstance(cmd, list):
        cmd = [
            c.replace("--enable-ldw-opt=false", "--enable-ldw-opt=true")
            if isinstance(c, str) else c
            for c in cmd
        ]
    return _orig_run_command(cmd, *a, **kw)


def _cached_compile_bir_kernel(bir_json, tmpdir, neff_name="file.neff"):
    try:
        key = hashlib.sha256(
            (b"ldw1" if _LDW_OPT else b"ldw0")
            + (bir_json if isinstance(bir_json, bytes) else bir_json.encode())
        ).hexdigest()[:24]
        os.makedirs(_NEFF_CACHE_DIR, exist_ok=True)
        cached = os.path.join(_NEFF_CACHE_DIR, key + ".neff")
        if os.path.exists(cached):
            dst = os.path.join(tmpdir, neff_name)
            shutil.copy(cached, dst)
            return dst
    except Exception:
        cached = None
    _bass_utils.run_command = _ldwopt_run_command
    try:
        neff_path = _orig_compile_bir_kernel(bir_json, tmpdir, neff_name)
    finally:
        _bass_utils.run_command = _orig_run_command
    try:
        if cached:
            shutil.copy(neff_path, cached)
    except Exception:
        pass
    return neff_path


_bass_utils.compile_bir_kernel = _cached_compile_bir_kernel
_bass2jax.compile_bir_kernel = _cached_compile_bir_kernel

# ---------------------------------------------------------------------------
# Problem constants (hardcoded per contract)
# ---------------------------------------------------------------------------
B = 4096
N = 2 * B          # 8192 rows
D = 128
P = 128
NCORES = 8
BLK = N // NCORES  # 1024 rows per core
NM = BLK // P      # 8 M-tiles
TAU = 0.1
ALPHA = 10.0 / float(np.log(2.0))   # sim tiles hold t = alpha*s
LN2 = float(np.log(2.0))
MAGIC_A = float(2.0 ** 23)
MAGIC_B = float(127 * 2 ** 23 - 472907)  # mean-calibrated Schraudolph bias

SIMW = 2048        # sim PSUM tile width (4 banks)
NSIM = N // SIMW   # 4 col tiles per M row
QW = 512           # matmul moving width
ST = 16            # norm groups per slab (slab = 2048 rows)

f32 = mybir.dt.float32
bf16 = mybir.dt.bfloat16
i32 = mybir.dt.int32

# exp tile assignment; linear idx = m*4 + nb.  DVE tiles use the 2-op
# Schraudolph path; POOL tiles use gpsimd vpowf STT; rest go to ACT.
DVE_SET = frozenset({3, 7, 11, 19, 23, 27})
POOL_SET = frozenset()

_cached_nc = None


def _build_nc():
    nc = bass.Bass()
    xT = nc.declare_dram_parameter("xT", [P, N], bf16, isOutput=False)
    xR = nc.declare_dram_parameter("xR", [P, N // P, D], bf16, isOutput=False)
    ident = nc.declare_dram_parameter("ident", [P, P], f32, isOutput=False)
    bigi = nc.declare_dram_parameter("bigi", [P, P], f32, isOutput=False)
    outrs = nc.declare_dram_parameter("outrs", [P, NM * NSIM], f32, isOutput=True)
    outrs2 = nc.declare_dram_parameter("outrs2", [P, NM * NSIM], f32, isOutput=True)
    outpos = nc.declare_dram_parameter("outpos", [P, NM], f32, isOutput=True)
    scratch = nc.dram_tensor("scratch", [N // P, P], bf16)[:, :]

    with tile.TileContext(nc) as tc, ExitStack() as ctx:
        const = ctx.enter_context(tc.tile_pool(name="const", bufs=1))
        xtp = ctx.enter_context(tc.tile_pool(name="xtp", bufs=1))
        xrp = ctx.enter_context(tc.tile_pool(name="xrp", bufs=2))
        sqp = ctx.enter_context(tc.tile_pool(name="sqp", bufs=2))
        nrmp = ctx.enter_context(tc.tile_pool(name="nrmp", bufs=2))
        invp = ctx.enter_context(tc.tile_pool(name="invp", bufs=1))
        rnp = ctx.enter_context(tc.tile_pool(name="rnp", bufs=1))
        ep = ctx.enter_context(tc.tile_pool(name="ep", bufs=2))
        ed = ctx.enter_context(tc.tile_pool(name="ed", bufs=2))
        ttrp = ctx.enter_context(tc.tile_pool(name="ttrp", bufs=2))
        tailp = ctx.enter_context(tc.tile_pool(name="tailp", bufs=1))
        simp = ctx.enter_context(tc.tile_pool(name="sim", bufs=2, space="PSUM"))

        # --- input loads first: the norm chain blocks on xR slab 0, so it
        # leads its queue; constants follow the inputs.
        xt_tiles = []
        xr_tiles = []
        xr_view = xR  # [128, 64, 128]
        for s in range(NSIM):
            xr_t = xrp.tile([P, ST, D], bf16, tag=f"xr{s}")
            (nc.sync if s % 2 == 0 else nc.gpsimd).dma_start(
                out=xr_t, in_=xr_view[:, s * ST:(s + 1) * ST, :]
            )
            xr_tiles.append(xr_t)
        xt_queues = [nc.scalar, nc.sync, nc.gpsimd, nc.sync]
        for s in range(NSIM):
            xt_t = xtp.tile([P, SIMW], bf16, tag=f"xt{s}")
            xt_queues[s].dma_start(out=xt_t, in_=xT[:, s * SIMW:(s + 1) * SIMW])
            xt_tiles.append(xt_t)

        # --- constants ---
        id_sb = const.tile([P, P], f32)
        nc.sync.dma_start(out=id_sb, in_=ident[:, :])
        bigI = const.tile([P, P], f32)
        nc.sync.dma_start(out=bigI, in_=bigi[:, :])
        twos = None
        if POOL_SET:
            twos = const.tile([P, SIMW], bf16)
            nc.vector.memset(twos, 2.0)

        # --- norms (compact [128, 16] per slab):
        # sq (DVE 2x) -> group-reduce (DVE) -> sqrt (ACT, all four emitted
        # before any Exp so the act table loads exactly twice) -> reciprocal
        # (DVE) -> bf16 -> PE transpose -> DRAM bounce -> stride-0 partition
        # broadcast -> rn = xT * inv (DVE 2x) ---
        rn_tiles = []
        inv_rep = invp.tile([P, N], bf16)
        for s in range(NSIM):
            sq_t = sqp.tile([P, ST, D], bf16, tag="sq")
            nc.vector.tensor_tensor(
                out=sq_t, in0=xr_tiles[s], in1=xr_tiles[s],
                op=mybir.AluOpType.mult,
            )
            n2_s = nrmp.tile([P, ST], f32, tag="n2")
            nc.vector.tensor_reduce(
                out=n2_s, in_=sq_t, axis=mybir.AxisListType.X,
                op=mybir.AluOpType.add,
            )
            nrm_s = nrmp.tile([P, ST], f32, tag="nrm")
            nc.scalar.activation(
                out=nrm_s, in_=n2_s, func=mybir.ActivationFunctionType.Sqrt
            )
            inv_s = nrmp.tile([P, ST], f32, tag="inv")
            nc.vector.reciprocal(out=inv_s, in_=nrm_s)

            # transpose via PE into a corner of a rotating psum tile
            tp_ps = simp.tile([P, SIMW], f32, tag="sim")
            nc.tensor.transpose(tp_ps[0:ST, 0:P], inv_s[:, :], id_sb[:, :])
            invT = nrmp.tile([ST, P], bf16, tag="invT")
            nc.vector.tensor_copy(out=invT, in_=tp_ps[0:ST, 0:P])
            nc.sync.dma_start(out=scratch[s * ST:(s + 1) * ST, :], in_=invT)
            seg = bass.AP(
                tensor=scratch.tensor,
                offset=scratch.offset + s * SIMW,
                ap=[[0, P], [1, SIMW]],
            )
            nc.gpsimd.dma_start(out=inv_rep[:, s * SIMW:(s + 1) * SIMW], in_=seg)

            rn_t = rnp.tile([P, SIMW], bf16, tag=f"rn{s}")
            nc.vector.tensor_tensor(
                out=rn_t, in0=xt_tiles[s],
                in1=inv_rep[:, s * SIMW:(s + 1) * SIMW],
                op=mybir.AluOpType.mult,
            )
            rn_tiles.append(rn_t)

        # alpha-scaled lhsT (first 1024 cols = this core's row block)
        rns = const.tile([P, BLK], bf16)
        nc.vector.tensor_scalar(
            out=rns, in0=rn_tiles[0][:, 0:BLK], scalar1=ALPHA, scalar2=None,
            op0=mybir.AluOpType.mult,
        )

        # --- gram + split exp/rowsum ---
        # per-engine accum tiles so ACT/DVE/Pool never write the same tile
        rs_cols = tailp.tile([P, NM * NSIM], f32)
        rs_dve = tailp.tile([P, NM * NSIM], f32)
        pos_all = tailp.tile([P, NM], f32)
        for m in range(NM):
            lhsT = rns[:, m * P:(m + 1) * P]
            for nb in range(NSIM):
                idx = m * NSIM + nb
                simt = simp.tile([P, SIMW], f32, tag="sim")
                for q in range(SIMW // QW):
                    nc.tensor.matmul(
                        simt[:, q * QW:(q + 1) * QW], lhsT,
                        rn_tiles[nb][:, q * QW:(q + 1) * QW],
                        start=True, stop=True,
                    )
                if nb == 0:
                    # kill self-sim: t - 100 -> 2^(t-100) ~ 0
                    nc.vector.tensor_tensor(
                        out=simt[:, m * P:(m + 1) * P],
                        in0=simt[:, m * P:(m + 1) * P],
                        in1=bigI, op=mybir.AluOpType.subtract,
                    )
                if nb == 2:
                    # positives: diagonal at free offset m*128 (tile covers
                    # cols 4096..6144), psum value = alpha * s_pos (exact)
                    pscr = ttrp.tile([P, P], f32, tag="ttr")
                    nc.vector.tensor_tensor(
                        out=pscr, in0=simt[:, m * P:(m + 1) * P],
                        in1=id_sb, op=mybir.AluOpType.mult,
                    )
                    nc.vector.tensor_reduce(
                        out=pos_all[:, m:m + 1], in_=pscr,
                        axis=mybir.AxisListType.X, op=mybir.AluOpType.add,
                    )
                if idx in DVE_SET:
                    ti = ed.tile([P, SIMW], i32, tag="ed")
                    nc.vector.tensor_scalar(
                        out=ti, in0=simt, scalar1=MAGIC_A, scalar2=MAGIC_B,
                        op0=mybir.AluOpType.mult, op1=mybir.AluOpType.add,
                    )
                    nc.vector.tensor_reduce(
                        out=rs_dve[:, idx:idx + 1], in_=ti[:, :].bitcast(f32),
                        axis=mybir.AxisListType.X, op=mybir.AluOpType.add,
                    )
                elif idx in POOL_SET:
                    e_t = ep.tile([P, SIMW], bf16, tag="eg")
                    nc.gpsimd.scalar_tensor_tensor(
                        out=e_t, in0=twos, scalar=0.0, in1=simt,
                        op0=mybir.AluOpType.bypass, op1=mybir.AluOpType.pow,
                        accum_out=rs_cols[:, idx:idx + 1],
                    )
                else:
                    e_t = ep.tile([P, SIMW], bf16, tag="ep")
                    nc.scalar.activation(
                        out=e_t, in_=simt,
                        func=mybir.ActivationFunctionType.Exp, scale=LN2,
                        accum_out=rs_cols[:, idx:idx + 1],
                    )

        nc.sync.dma_start(out=outrs[:, :], in_=rs_cols)
        nc.sync.dma_start(out=outrs2[:, :], in_=rs_dve)
        nc.sync.dma_start(out=outpos[:, :], in_=pos_all)

    return nc


# Test/profiling hooks (unused by the grading path: TRACE defaults False).
TRACE = False
TRACE_DIR = None
LAST_RESULTS = None


def kernel(zis, zjs):
    global _cached_nc, LAST_RESULTS
    if _cached_nc is None:
        _cached_nc = _build_nc()
    nc = _cached_nc

    zis = np.asarray(zis, dtype=np.float32)
    zjs = np.asarray(zjs, dtype=np.float32)
    reps = np.concatenate([zjs, zis], axis=0)  # [8192, 128]

    id_h = np.eye(P, dtype=np.float32)
    bigi_h = (100.0 * np.eye(P)).astype(np.float32)
    in_maps = []
    for c in range(NCORES):
        rot = np.roll(reps, -BLK * c, axis=0)
        rot16 = rot.astype(ml_dtypes.bfloat16)
        in_maps.append(
            {
                "xT": np.ascontiguousarray(rot16.T),
                # [128, 64, 128]: row r = 128*t + p -> [p, t, :]
                "xR": np.ascontiguousarray(
                    rot16.reshape(N // P, P, D).transpose(1, 0, 2)
                ),
                "ident": id_h,
                "bigi": bigi_h,
            }
        )

    kwargs = {}
    if TRACE:
        kwargs = dict(trace=True, tmpdir=TRACE_DIR)
    res = run_bass_kernel_spmd(nc, in_maps, list(range(NCORES)), **kwargs)
    LAST_RESULTS = res

    # host tail in f64: den per row, then CE / pt partials
    ce_total = 0.0
    pt_total = 0.0
    dve_cols = np.array([i in DVE_SET for i in range(NM * NSIM)])
    for r in res.results:
        rs_a = np.asarray(r["outrs"], np.float64)     # [128, 32] ACT/Pool
        rs_d = np.asarray(r["outrs2"], np.float64)    # [128, 32] DVE
        rs = np.where(dve_cols[None, :], rs_d, rs_a)
        posv = np.asarray(r["outpos"], np.float64)    # [128, 8] = alpha*s_pos
        den = rs.reshape(P, NM, NSIM).sum(axis=2)     # [128, 8]
        logden = np.log(den)
        ce_total += (logden - LN2 * posv).sum()
        pt_total += (np.exp2(posv) / den).sum()

    n = float(N)
    b = float(B)
    loss = ce_total / n + b * (1.0 / b - pt_total / (n * (n - 1.0)))
    return np.float32(loss)



# revision 2
# speedup vs baseline: 1.6162x; 1.6162x over previous
"""Trainium2 Bass kernel for nn_ContrastiveLoss (SimCLR NT-Xent) — v3.

Math (reference):
    reps = concat(zjs, zis)            # [8192, 128]
    rn = reps / ||reps||               # row-normalized
    sim = rn @ rn.T                    # [8192, 8192]
    den_i = sum_{j != i} exp(sim[i,j]/tau);  pos_i = sim[i, i+-B]
    CE = sum_i (log den_i - pos_i/tau);  pt = sum_i exp(pos_i/tau)/den_i
    loss = CE/N + B*(1/B - pt/(N*(N-1)))

v3 design (vs the 114us v2 baseline):
  * host-side normalization: rn computed in numpy, shipped as bf16.  The
    entire on-device norm chain (square/reduce/sqrt/recip/transpose/DMA
    bounce) is gone, so matmuls start ~2us in and the PE HAM clock-gate
    warms early (plus explicit dummy warm-up matmuls during the DMA fill).
  * symmetry: sim is symmetric, so each core computes only 6 of 8 column
    super-blocks (rows own block c, cols blocks c..c+5).  The two skipped
    blocks' den contributions are recovered from COLUMN sums of the o=1,2
    blocks via ones-vector matmuls on the PE (cheap), accumulated over m
    in a PSUM acc tile and combined on the host.  25% less exp work.
  * exp split ACT/DVE by tile: ACT runs Exp(scale=ln2) with accum_out row
    sums; DVE runs an int16 Schraudolph (t*2^7 + magic -> bitcast bf16)
    then a 2x-rate bf16 row-sum reduce.
  * positives and the scalar log/exp tail are host-side (f64).
"""
import os

os.environ.setdefault("JAX_COMPILATION_CACHE_DIR", "/root/jax_bass_cache")

import numpy as np
import ml_dtypes
from contextlib import ExitStack

import concourse.bass as bass
import concourse.tile as tile
from concourse import mybir
from concourse.bass_utils import run_bass_kernel_spmd
from concourse.vector_clock import ScopedClock

# ---------------------------------------------------------------------------
# Workaround for walrus CoreV2/V3 "Too many sync wait commands": split sem
# waits so no instruction carries more than one.
# ---------------------------------------------------------------------------
_MAX_WAITS = int(os.environ.get("BASS_MAX_WAITS", "1"))
_orig_commit = tile.TileContext._commit_instruction


def _split_waits(nc, inst):
    si = getattr(inst, "sync_info", None)
    if si is None:
        return []
    waits = list(si.on_wait)
    if len(waits) <= _MAX_WAITS:
        return []
    nops = []
    excess, keep = waits[:-_MAX_WAITS], waits[-_MAX_WAITS:]
    for i in range(0, len(excess), _MAX_WAITS):
        nops.append(
            mybir.InstNoOp(
                name=nc.get_next_instruction_name(),
                engine=inst.engine,
                bass_nofuse=True,
                sync_info=mybir.SyncInfo(
                    on_wait=excess[i : i + _MAX_WAITS], on_update=[]
                ),
            )
        )
    inst.sync_info = mybir.SyncInfo(on_wait=keep, on_update=list(si.on_update))
    return nops


def _patched_commit(self, inst, lazy_reg_writes=True):
    try:
        nops = _split_waits(self.nc, inst)
    except Exception:
        nops = []
    for nop in nops:
        _orig_commit(self, nop)
    return _orig_commit(self, inst, lazy_reg_writes)


def _patched_drain_and_barrier(self, tick_clock, wait_clock):
    nc = self.nc
    probe = mybir.InstNoOp(
        name=nc.get_next_instruction_name(),
        engine=mybir.EngineType.SP,
        bass_nofuse=True,
    )
    wait_clock.add_sem_waits(probe, ScopedClock({None: tick_clock.global_clock}))
    si = probe.sync_info
    waits = list(si.on_wait) if si is not None else []
    for i in range(0, len(waits), _MAX_WAITS):
        nop = nc.sync.nop(nofuse=True)
        nop.ins.sync_info = mybir.SyncInfo(
            on_wait=waits[i : i + _MAX_WAITS], on_update=[]
        )
    nc.sync.drain()
    nc.all_engine_barrier()
    assert self.sems is not None
    popped = nc._tile_sem_poison_stack.pop()
    assert popped is self._sem_poison
    nc.clear_and_free_semaphores(list(self.sems.allocated().values()))
    nc.all_engine_barrier()


tile.TileContext._commit_instruction = _patched_commit
tile.TileContext._drain_and_barrier = _patched_drain_and_barrier

# ---------------------------------------------------------------------------
# Content-hashed NEFF cache
# ---------------------------------------------------------------------------
import hashlib
import shutil

_NEFF_CACHE_DIR = "/root/.bass_neff_cache"

import concourse.bass_utils as _bass_utils
import concourse.bass2jax as _bass2jax

_orig_compile_bir_kernel = _bass_utils.compile_bir_kernel


_LDW_OPT = os.environ.get("BASS_LDW_OPT", "0") == "1"
_orig_run_command = _bass_utils.run_command


def _ldwopt_run_command(cmd, *a, **kw):
    if _LDW_OPT and isinstance(cmd, list):
        cmd = [
            c.replace("--enable-ldw-opt=false", "--enable-ldw-opt=true")
            if isinstance(c, str) else c
            for c in cmd
        ]
    return _orig_run_command(cmd, *a, **kw)


def _cached_compile_bir_kernel(bir_json, tmpdir, neff_name="file.neff"):
    try:
        key = hashlib.sha256(
            (b"ldw1" if _LDW_OPT else b"ldw0")
            + (bir_json if isinstance(bir_json, bytes) else bir_json.encode())
        ).hexdigest()[:24]
        os.makedirs(_NEFF_CACHE_DIR, exist_ok=True)
        cached = os.path.join(_NEFF_CACHE_DIR, key + ".neff")
        if os.path.exists(cached):
            dst = os.path.join(tmpdir, neff_name)
            shutil.copy(cached, dst)
            return dst
    except Exception:
        cached = None
    _bass_utils.run_command = _ldwopt_run_command
    try:
        neff_path = _orig_compile_bir_kernel(bir_json, tmpdir, neff_name)
    finally:
        _bass_utils.run_command = _orig_run_command
    try:
        if cached:
            shutil.copy(neff_path, cached)
    except Exception:
        pass
    return neff_path


_bass_utils.compile_bir_kernel = _cached_compile_bir_kernel
_bass2jax.compile_bir_kernel = _cached_compile_bir_kernel

# ---------------------------------------------------------------------------
# Problem constants (hardcoded per contract)
# ---------------------------------------------------------------------------
B = 4096
N = 2 * B          # 8192 rows
D = 128
P = 128
NCORES = 8
BLK = N // NCORES  # 1024 rows per core
NM = BLK // P      # 8 M-tiles
TAU = 0.1
ALPHA = 10.0 / float(np.log(2.0))   # sim tiles hold t = alpha*s
LN2 = float(np.log(2.0))

NSUP = 6           # column super-blocks computed (of 8); 2 come via symmetry
TW = 1536          # PSUM gram tile width (3 banks)
NT = 4             # tiles per m-row: 4*1536 = 6144 = NSUP*1024
QW = 512           # matmul moving width
# int16 Schraudolph: i16(t*2^7 + MAGIC16) bitcast as bf16 gives ~2^t
MAGIC16 = float(127 * 128) - 472907.0 / 65536.0

f32 = mybir.dt.float32
bf16 = mybir.dt.bfloat16
i16 = mybir.dt.int16

# tile engine assignment; idx = m*NT + t.  DVE tiles use the int16
# Schraudolph; the rest use ACT Exp with accum row-sums.
DVE_SET = frozenset(
    {m * NT + 0 for m in range(NM)} | {m * NT + 3 for m in range(0, NM, 2)}
)

_cached_nc = None


def _build_nc():
    nc = bass.Bass()
    xTp = [
        nc.declare_dram_parameter(f"xT{t}", [P, TW], bf16, isOutput=False)
        for t in range(NT)
    ]
    lhsT = nc.declare_dram_parameter("lhsT", [P, BLK], bf16, isOutput=False)
    onesw = nc.declare_dram_parameter("onesw", [P, 1], bf16, isOutput=False)
    bigi = nc.declare_dram_parameter("bigi", [P, P], f32, isOutput=False)
    outrsA = nc.declare_dram_parameter("outrsA", [P, NM * NT], f32, isOutput=True)
    outrsD = nc.declare_dram_parameter("outrsD", [P, NM * NT], f32, isOutput=True)
    outcs = nc.declare_dram_parameter("outcs", [33, BLK], f32, isOutput=True)

    with tile.TileContext(nc) as tc, ExitStack() as ctx:
        const = ctx.enter_context(tc.tile_pool(name="const", bufs=1))
        xtp = ctx.enter_context(tc.tile_pool(name="xtp", bufs=1))
        ep = ctx.enter_context(tc.tile_pool(name="ep", bufs=6))
        rsp = ctx.enter_context(tc.tile_pool(name="rsp", bufs=1))
        simp = ctx.enter_context(tc.tile_pool(name="sim", bufs=2, space="PSUM"))
        accp = ctx.enter_context(tc.tile_pool(name="acc", bufs=1, space="PSUM"))

        # --- warm-up constants first: DVE memset unblocks PE dummy matmuls
        # and the ACT table-load probe while input DMAs stream. ---
        warm = const.tile([P, QW], bf16)
        nc.vector.memset(warm, 0.125)
        trash = const.tile([P, 1], bf16)
        nc.scalar.activation(
            out=trash, in_=warm[:, 0:1],
            func=mybir.ActivationFunctionType.Exp, scale=LN2,
        )

        # --- input DMAs.  sync queue: lhsT + even xT tiles (needed first);
        # gpsimd queue: bigi/odd xT tiles/ones. ---
        lhs_sb = const.tile([P, BLK], bf16)
        nc.sync.dma_start(out=lhs_sb, in_=lhsT[:, :])
        bigi_sb = const.tile([P, P], f32)
        nc.gpsimd.dma_start(out=bigi_sb, in_=bigi[:, :])
        xt_sb = []
        for t in range(NT):
            xt_t = xtp.tile([P, TW], bf16, tag=f"xt{t}")
            (nc.sync if t % 2 == 0 else nc.gpsimd).dma_start(
                out=xt_t, in_=xTp[t][:, :]
            )
            xt_sb.append(xt_t)
        ones_sb = const.tile([P, 1], bf16)
        nc.gpsimd.dma_start(out=ones_sb, in_=onesw[:, :])

        # --- rowsum accumulators + colsum acc (memset before use) ---
        rsA = rsp.tile([P, NM * NT], f32)
        rsD = rsp.tile([P, NM * NT], f32)
        nc.vector.memset(rsA, 0.0)
        nc.vector.memset(rsD, 0.0)
        acc = accp.tile([P, BLK], f32, tag="acc")
        nc.vector.memset(acc[0:33, :], 0.0)

        # --- PE HAM warm-up: dummy matmuls on the memset const while the
        # real inputs are still in flight. ---
        warm_ps = simp.tile([P, TW], f32, tag="sim")
        for q in range(3):
            nc.tensor.matmul(
                warm_ps[:, q * QW:(q + 1) * QW], warm[:, 0:P], warm,
                start=True, stop=True,
            )

        # colsum chunks: (tile t, src off, acc partition, acc col off)
        CS_CHUNKS = {
            0: [(1024, 0, 0)],
            1: [(0, 0, 512), (512, 32, 0), (1024, 32, 512)],
        }

        def flush_colsums(items):
            for (e_t, m, t) in items:
                for (off, apart, acol) in CS_CHUNKS[t]:
                    nc.tensor.matmul(
                        acc[apart:apart + 1, acol:acol + QW],
                        ones_sb,
                        e_t[:, off:off + QW],
                        start=(m == 0), stop=(m == NM - 1),
                        tile_position=(0, apart),
                    )

        pending = []
        for m in range(NM):
            lhs_m = lhs_sb[:, m * P:(m + 1) * P]
            for t in range(NT):
                idx = m * NT + t
                sim_t = simp.tile([P, TW], f32, tag="sim")
                for q in range(3):
                    nc.tensor.matmul(
                        sim_t[:, q * QW:(q + 1) * QW], lhs_m,
                        xt_sb[t][:, q * QW:(q + 1) * QW],
                        start=True, stop=True,
                    )
                if t == 0 and pending:
                    # colsums for m-1's exp tiles; their exps are long done
                    # by the time the PE reaches these in its queue.
                    flush_colsums(pending)
                    pending = []
                if t == 0:
                    # kill self-sim: t - 100 -> 2^(t-100) ~ 0
                    nc.vector.tensor_tensor(
                        out=sim_t[:, m * P:(m + 1) * P],
                        in0=sim_t[:, m * P:(m + 1) * P],
                        in1=bigi_sb, op=mybir.AluOpType.subtract,
                    )
                e_t = ep.tile([P, TW], bf16, tag="ep")
                if idx in DVE_SET:
                    nc.vector.tensor_scalar(
                        out=e_t[:, :].bitcast(i16), in0=sim_t,
                        scalar1=128.0, scalar2=MAGIC16,
                        op0=mybir.AluOpType.mult, op1=mybir.AluOpType.add,
                    )
                    nc.vector.tensor_reduce(
                        out=rsD[:, idx:idx + 1], in_=e_t,
                        axis=mybir.AxisListType.X, op=mybir.AluOpType.add,
                    )
                else:
                    nc.scalar.activation(
                        out=e_t, in_=sim_t,
                        func=mybir.ActivationFunctionType.Exp, scale=LN2,
                        accum_out=rsA[:, idx:idx + 1],
                    )
                if t in CS_CHUNKS:
                    pending.append((e_t, m, t))

        flush_colsums(pending)

        # --- tail: evacuate colsum acc (partitions 0 and 32 are live) ---
        evac = rsp.tile([33, BLK], f32)
        nc.vector.tensor_copy(out=evac, in_=acc[0:33, :])
        nc.sync.dma_start(out=outrsA[:, :], in_=rsA)
        nc.sync.dma_start(out=outrsD[:, :], in_=rsD)
        nc.sync.dma_start(out=outcs[:, :], in_=evac)

    return nc


# Test/profiling hooks (unused by the grading path: TRACE defaults False).
TRACE = False
TRACE_DIR = None
LAST_RESULTS = None


def kernel(zis, zjs):
    global _cached_nc, LAST_RESULTS
    if _cached_nc is None:
        _cached_nc = _build_nc()
    nc = _cached_nc

    zis = np.asarray(zis, dtype=np.float32)
    zjs = np.asarray(zjs, dtype=np.float32)
    reps = np.concatenate([zjs, zis], axis=0)  # [8192, 128]
    nrm = np.maximum(np.linalg.norm(reps, axis=1, keepdims=True), 1e-8)
    rn32 = reps / nrm
    rn_bf = rn32.astype(ml_dtypes.bfloat16)

    ones_h = np.ones((P, 1), dtype=ml_dtypes.bfloat16)
    bigi_h = (100.0 * np.eye(P)).astype(np.float32)
    in_maps = []
    for c in range(NCORES):
        rot = np.roll(np.arange(N), -BLK * c)
        xTfull = np.ascontiguousarray(rn_bf[rot[:NSUP * BLK]].T)  # [128, 6144]
        im = {
            "lhsT": np.ascontiguousarray(
                (ALPHA * rn32[rot[:BLK]]).astype(ml_dtypes.bfloat16).T
            ),
            "onesw": ones_h,
            "bigi": bigi_h,
        }
        for t in range(NT):
            im[f"xT{t}"] = np.ascontiguousarray(xTfull[:, t * TW:(t + 1) * TW])
        in_maps.append(im)

    kwargs = {}
    if TRACE:
        kwargs = dict(trace=True, tmpdir=TRACE_DIR)
    res = run_bass_kernel_spmd(nc, in_maps, list(range(NCORES)), **kwargs)
    LAST_RESULTS = res

    # --- host tail in f64 ---
    dve_cols = np.array([i in DVE_SET for i in range(NM * NT)])
    den = np.zeros(N, dtype=np.float64)
    ar = np.arange(BLK)
    for c, r in enumerate(res.results):
        rs_a = np.asarray(r["outrsA"], np.float64)
        rs_d = np.asarray(r["outrsD"], np.float64)
        rs = np.where(dve_cols[None, :], rs_d, rs_a)          # [128, 32]
        rows = rs.reshape(P, NM, NT).sum(axis=2).T.reshape(-1)  # dev row m*128+p
        den[(ar + BLK * c) % N] += rows
        cs = np.asarray(r["outcs"], np.float64)               # [33, 1024]
        den[(ar + BLK * (c + 1)) % N] += cs[0]
        den[(ar + BLK * (c + 2)) % N] += cs[32]

    idx = np.arange(N)
    pos_idx = np.where(idx < B, idx + B, idx - B)
    rnh = rn_bf.astype(np.float64)
    posv = (rnh * rnh[pos_idx]).sum(axis=1)

    n = float(N)
    b = float(B)
    CE = (np.log(den) - 10.0 * posv).sum()
    pt = (np.exp(10.0 * posv) / den).sum()
    loss = CE / n + b * (1.0 / b - pt / (n * (n - 1.0)))
    return np.float32(loss)
